# revision 17
# baseline (speedup 1.0000x reference)
"""Trainium2 Bass kernel for nn_MultiHeadDotProductAttention (b=4, L=2048,
d_model=1024, 16 heads x 64 head_dim, additive attention bias, softmax).

Sharding: 8 cores = 2 batch-groups (2 batches each) x 4 head-groups (4 heads
each). Each core computes, for its 2 batches and 4 heads, the full attention
pipeline and an output-projection PARTIAL (summed over its 4 heads); the host
sums the 4 head-group partials per batch and adds the output bias.

Device layout ("T layout"): everything keeps sequence-length on the free dim
and feature dims on partitions, so no on-device transposes are needed:
  qT,kT: [hd, l]   from  out = wq^T @ xT  (xT transposed on host)
  logitsT[lk, lq] = kT^T-slices (K=64 contraction, two heads row-packed in the
                    128x128 PE array via tile_position)
  softmax over lk: exp on ACT; denominators via a ones-column appended to V in
                   the AV matmul (col-packed via tile_position); normalization
                   by reciprocal + gpsimd partition_broadcast + DVE multiply.
  out = ctxT^T @ wo with ctxT [hd, lq] directly produced by AV.

All matmuls run in float32r (tf32-class, 1 cycle/row at free-dim >= 256).
The attention bias is streamed as pre-transposed bf16 (biasT[h, lk, lq]) and
added to the fp32 logits in PSUM on the DVE.
"""

import numpy as np
from contextlib import ExitStack

import ml_dtypes

import concourse.bass as bass
import concourse.mybir as mybir
import concourse.tile as tile
from concourse import bacc
from concourse import bass_utils

F32 = mybir.dt.float32
F32R = mybir.dt.float32r
BF16 = mybir.dt.bfloat16
AF = mybir.ActivationFunctionType

# ---- problem constants (hardcoded per contract) ----
B, L, D = 4, 2048, 1024
H, DH = 16, 64
NB = 2          # batch groups (batches per core = B // NB = 2)
NH = 4          # head groups  (heads per core = H // NH = 4)
BPC = B // NB   # 2 batches per core
HPC = H // NH   # 4 heads per core
PAIRS = HPC // 2
KSUB = D // 128          # 8 contraction subtiles for projections
LCH = 256                # x-stream chunk width (free dim of projection mms)
NLC = L // LCH           # 8 chunks
NQ = 4                   # lq chunks of 512 for attention
NI = 16                  # lk chunks of 128
HD = HPC * DH            # 256 local head dims
HDC = HD // 128          # 2 local hd chunks (= PAIRS)

# bias dtype streamed to the device ("bf16" or "f32")
BIAS_DT = "bf16"
# fraction control: every PE_BIAS_EVERYth i-index adds bias on the PE via an
# identity matmul instead of the DVE (0 = never)
PE_BIAS_EVERY = 1

DEBUG_DUMPS = False

_CACHED = {}


def _build_bass():
    nc = bacc.Bacc("TRN2", target_bir_lowering=False, debug=False, num_devices=8)

    bias_dt = BF16 if BIAS_DT == "bf16" else F32R

    # ---- DRAM I/O (per core) ----
    xq_d = nc.dram_tensor("xq_t", [BPC, D, L], BF16, kind="ExternalInput")
    xk_d = nc.dram_tensor("xk_t", [BPC, D, L], BF16, kind="ExternalInput")
    bias_d = nc.dram_tensor("bias_t", [HPC, L, L], bias_dt, kind="ExternalInput")
    wq_d = nc.dram_tensor("wq", [D, HD], BF16, kind="ExternalInput")
    wk_d = nc.dram_tensor("wk", [D, HD], BF16, kind="ExternalInput")
    wv_d = nc.dram_tensor("wv", [D, HD], BF16, kind="ExternalInput")
    wo_d = nc.dram_tensor("wo", [HD, D], BF16, kind="ExternalInput")
    bq_d = nc.dram_tensor("bq", [HD], F32, kind="ExternalInput")
    bk_d = nc.dram_tensor("bk", [HD], F32, kind="ExternalInput")
    bv_d = nc.dram_tensor("bv", [HD], BF16, kind="ExternalInput")
    out_d = nc.dram_tensor("out_part", [BPC, L, D], F32, kind="ExternalOutput")
    if DEBUG_DUMPS:
        qT_d = nc.dram_tensor("qT_dbg", [128, HDC, BPC, L], F32R, kind="ExternalOutput")
        kT_d = nc.dram_tensor("kT_dbg", [128, HDC, BPC, L], F32R, kind="ExternalOutput")
        v_d = nc.dram_tensor("v_dbg", [128, NI, BPC, HPC, DH + 1], F32R, kind="ExternalOutput")
        ctxT_d = nc.dram_tensor("ctxT_dbg", [128, HDC, BPC, L], F32R, kind="ExternalOutput")
        exp_d = nc.dram_tensor("exp_dbg", [128, 512], F32R, kind="ExternalOutput")
        av_d = nc.dram_tensor("av_dbg", [128, 512], F32, kind="ExternalOutput")

    with tile.TileContext(nc) as tc, ExitStack() as top:
        # ---- persistent SBUF ----
        pers = top.enter_context(tc.tile_pool(name="pers", bufs=1))
        qT = pers.tile([128, HDC, BPC, L], BF16)
        kT = pers.tile([128, HDC, BPC, L], BF16)
        v = pers.tile([128, NI, BPC, HPC, DH + 1], BF16)
        ctxT = pers.tile([128, HDC, BPC, L], BF16)
        wo_s = pers.tile([128, HDC, D], BF16)
        bq_s = pers.tile([128, HDC], F32)
        bk_s = pers.tile([128, HDC], F32)
        bv_row = pers.tile([1, HD], BF16)
        ones_col = pers.tile([1, 128], BF16)
        ones_r = pers.tile([128, 128], F32R)
        ident_bf = pers.tile([128, 128], BF16)

        nc.sync.dma_start(wo_s[:], wo_d.rearrange("(c p) n -> p c n", p=128))
        nc.sync.dma_start(bq_s[:], bq_d.rearrange("(c p) -> p c", p=128))
        nc.sync.dma_start(bk_s[:], bk_d.rearrange("(c p) -> p c", p=128))
        nc.sync.dma_start(bv_row[:], bv_d[None, :])
        ones_f32 = pers.tile([128, 128], F32)
        nc.vector.memset(ones_f32[:], 1.0)
        nc.vector.tensor_copy(ones_col[:], ones_f32[0:1, :])
        nc.vector.tensor_copy(ones_r[:], ones_f32[:])
        from concourse.masks import make_identity
        ident_f32 = pers.tile([128, 128], F32)
        make_identity(nc, ident_f32[:])
        nc.vector.tensor_copy(ident_bf[:], ident_f32[:])
        # softmax-denominator column of v (column DH is all-ones)
        nc.vector.tensor_copy(
            v[:, :, :, :, DH],
            ones_f32[:, 0:NI * BPC * HPC].rearrange(
                "p (a b c) -> p a b c", a=NI, b=BPC
            ),
        )

        # ---- P1: projections ----
        with ExitStack() as p1:
            wpool = p1.enter_context(tc.tile_pool(name="wqkv", bufs=1))
            wq_s = wpool.tile([128, KSUB, HD], BF16)
            wk_s = wpool.tile([128, KSUB, HD], BF16)
            wv_s = wpool.tile([128, KSUB, HD], BF16)
            nc.sync.dma_start(wq_s[:], wq_d.rearrange("(k p) n -> p k n", p=128))
            nc.sync.dma_start(wk_s[:], wk_d.rearrange("(k p) n -> p k n", p=128))
            nc.sync.dma_start(wv_s[:], wv_d.rearrange("(k p) n -> p k n", p=128))

            xpool = p1.enter_context(tc.tile_pool(name="xs", bufs=2))
            psq = p1.enter_context(tc.tile_pool(name="psq", bufs=3, space="PSUM"))
            psv = p1.enter_context(tc.tile_pool(name="psv", bufs=2, space="PSUM"))

            for b in range(BPC):
                xqr = xq_d[b].rearrange("(k p) l -> p k l", p=128)
                xkr = xk_d[b].rearrange("(k p) l -> p k l", p=128)
                for c in range(NLC):
                    sl = slice(c * LCH, (c + 1) * LCH)
                    xq_t = xpool.tile([128, KSUB, LCH], BF16, tag="xq")
                    xk_t = xpool.tile([128, KSUB, LCH], BF16, tag="xk")
                    nc.sync.dma_start(xq_t[:], xqr[:, :, sl])
                    nc.sync.dma_start(xk_t[:], xkr[:, :, sl])
                    # qT / kT: out[hd-chunk(128), lq-chunk] = wq^T @ xT
                    for m in range(HDC):
                        msl = slice(m * 128, (m + 1) * 128)
                        pq = psq.tile([128, LCH], F32, tag="ps")
                        for k in range(KSUB):
                            nc.tensor.matmul(
                                pq[:], wq_s[:, k, msl], xq_t[:, k, :],
                                start=(k == 0), stop=(k == KSUB - 1),
                            )
                        nc.scalar.activation(
                            qT[:, m, b, sl], pq[:], AF.Identity, bias=bq_s[:, m:m + 1]
                        )
                        pk = psq.tile([128, LCH], F32, tag="ps")
                        for k in range(KSUB):
                            nc.tensor.matmul(
                                pk[:], wk_s[:, k, msl], xk_t[:, k, :],
                                start=(k == 0), stop=(k == KSUB - 1),
                            )
                        nc.scalar.activation(
                            kT[:, m, b, sl], pk[:], AF.Identity, bias=bk_s[:, m:m + 1]
                        )
                    # v: out[lk-sub(128), hd(256)] = xT-slices^T @ wv  (+ bv row)
                    for s in range(LCH // 128):
                        si = c * (LCH // 128) + s
                        pv = psv.tile([128, HD], F32, tag="ps")
                        for k in range(KSUB):
                            nc.tensor.matmul(
                                pv[:], xk_t[:, k, s * 128:(s + 1) * 128],
                                wv_s[:, k, :],
                                start=(k == 0), stop=False,
                            )
                        nc.tensor.matmul(
                            pv[:], ones_col[:], bv_row[:], start=False, stop=True
                        )
                        nc.vector.tensor_copy(
                            v[:, si, b, :, 0:DH],
                            pv[:].rearrange("p (h d) -> p h d", h=HPC),
                        )

        # ---- P2: attention ----
        with ExitStack() as p2:
            bpool = p2.enter_context(tc.tile_pool(name="biasb", bufs=6))
            epool = p2.enter_context(tc.tile_pool(name="expb", bufs=6))
            rpool = p2.enter_context(tc.tile_pool(name="recip", bufs=2))
            scpool = p2.enter_context(tc.tile_pool(name="scsh", bufs=2))
            reppool = p2.enter_context(tc.tile_pool(name="rep", bufs=2))
            pslg = p2.enter_context(tc.tile_pool(name="pslg", bufs=4, space="PSUM"))
            psav = p2.enter_context(tc.tile_pool(name="psav", bufs=4, space="PSUM"))

            for p in range(PAIRS):
                for n in range(NQ):
                    nsl = slice(n * 512, (n + 1) * 512)
                    av = {}
                    for hl in range(2):
                        for b in range(BPC):
                            av_t = psav.tile([128, 512], F32, tag="av")
                            av[hl, b] = av_t
                    for i in range(NI):
                        isl = slice(i * 128, (i + 1) * 128)
                        for hl in range(2):
                            h = p * 2 + hl
                            rsl = slice(hl * 64, (hl + 1) * 64)
                            bias_t = bpool.tile([128, 512], bias_dt, tag="bias")
                            nc.sync.dma_start(bias_t[:], bias_d[h, isl, nsl])
                            for b in range(BPC):
                                pe_bias = True
                                lg = pslg.tile([128, 512], F32, tag="lg")
                                nc.tensor.matmul(
                                    lg[:],
                                    kT[rsl, p, b, isl],
                                    qT[rsl, p, b, nsl],
                                    start=True, stop=not pe_bias,
                                    tile_position=(hl * 64, 0),
                                )
                                if pe_bias:
                                    nc.tensor.matmul(
                                        lg[:], ident_bf[:], bias_t[:],
                                        start=False, stop=True,
                                        skip_group_check=True,
                                    )
                                else:
                                    nc.vector.tensor_add(lg[:], lg[:], bias_t[:])
                                et = epool.tile([128, 512], BF16, tag="exp")
                                nc.scalar.activation(et[:], lg[:], AF.Exp)
                                # AV accumulate: ctx rows + ones-row sums,
                                # col-packed so the pair lands at rows 0-63 /
                                # 64-127 of its psum (plus a sums row each).
                                if DEBUG_DUMPS and p == 0 and n == 0 and i == 0 and hl == 0 and b == 0:
                                    nc.sync.dma_start(exp_d[:], et[:])
                                nc.tensor.matmul(
                                    av[hl, b][0:DH + 1, :],
                                    v[:, i, b, 2 * p + hl, :],
                                    et[:],
                                    start=(i == 0), stop=(i == NI - 1),
                                )
                    if DEBUG_DUMPS and p == 0 and n == 0:
                        avdump = rpool.tile([128, 512], F32, tag="avd")
                        nc.vector.tensor_copy(avdump[:], av[0, 0][:])
                        nc.sync.dma_start(av_d[:], avdump[:])
                    # normalize -> ctxT: move sums row to SBUF, replicate
                    # across partitions with a K=1 ones matmul, reciprocal on
                    # DVE, then scale the ctx rows.
                    for hl in range(2):
                        for b in range(BPC):
                            rs = rpool.tile([128, 512], F32R, tag="rs")
                            nc.vector.tensor_copy(rs[64:65, :], av[hl, b][64:65, :])
                            repp = pslg.tile([128, 512], F32, tag="lg")
                            nc.tensor.matmul(
                                repp[:], ones_r[64:65, :], rs[64:65, :],
                                start=True, stop=True,
                            )
                            rep = reppool.tile([128, 512], F32, tag="rep")
                            rscr = reppool.tile([128, 512], F32, tag="rscr")
                            nc.vector.reciprocal_approx_accurate(
                                rep[0:64, :], repp[0:64, :], rscr[0:64, :]
                            )
                            if hl == 0:
                                nc.vector.tensor_mul(
                                    ctxT[0:64, p, b, nsl],
                                    av[hl, b][0:64, :],
                                    rep[0:64, :],
                                )
                            else:
                                sc = scpool.tile([64, 512], BF16, tag="sc")
                                nc.vector.tensor_mul(
                                    sc[:], av[hl, b][0:64, :], rep[0:64, :]
                                )
                                nc.sync.dma_start(ctxT[64:128, p, b, nsl], sc[:])

        if DEBUG_DUMPS:
            nc.sync.dma_start(qT_d[:], qT[:])
            nc.sync.dma_start(kT_d[:], kT[:])
            nc.sync.dma_start(v_d[:], v[:])
            nc.sync.dma_start(ctxT_d[:], ctxT[:])

        # ---- P3: output projection (partial over local heads) ----
        with ExitStack() as p3:
            opool = p3.enter_context(tc.tile_pool(name="outb", bufs=4))
            psout = p3.enter_context(tc.tile_pool(name="psout", bufs=2, space="PSUM"))
            for b in range(BPC):
                for m in range(L // 128):
                    msl = slice(m * 128, (m + 1) * 128)
                    for nn in range(D // 512):
                        osl = slice(nn * 512, (nn + 1) * 512)
                        po = psout.tile([128, 512], F32, tag="po")
                        for kc in range(HDC):
                            nc.tensor.matmul(
                                po[:],
                                ctxT[:, kc, b, msl],
                                wo_s[:, kc, osl],
                                start=(kc == 0), stop=(kc == HDC - 1),
                            )
                        ot = opool.tile([128, 512], F32, tag="ot")
                        nc.vector.tensor_copy(ot[:], po[:])
                        nc.sync.dma_start(out_d[b, msl, osl], ot[:])

    nc.compile()
    return nc


def make_in_maps(inputs_q, inputs_kv, bias, wq, bq, wk, bk, wv, bv, wo, bo):
    inputs_q = np.asarray(inputs_q, np.float32)
    inputs_kv = np.asarray(inputs_kv, np.float32)
    bias = np.asarray(bias, np.float32)
    wq = np.asarray(wq, np.float32).reshape(D, H * DH)
    wk = np.asarray(wk, np.float32).reshape(D, H * DH)
    wv = np.asarray(wv, np.float32).reshape(D, H * DH)
    bq = np.asarray(bq, np.float32).reshape(H * DH)
    bk = np.asarray(bk, np.float32).reshape(H * DH)
    bv = np.asarray(bv, np.float32).reshape(H * DH)
    wo = np.asarray(wo, np.float32).reshape(H * DH, D)
    bo = np.asarray(bo, np.float32)

    # fold the 1/sqrt(head_dim) query scaling into wq/bq
    s = 1.0 / np.sqrt(DH)
    wq = wq * s
    bq = bq * s

    # host-side layout marshalling for the chosen sharding
    xqT = np.ascontiguousarray(inputs_q.transpose(0, 2, 1)).astype(
        ml_dtypes.bfloat16
    )
    xkT = np.ascontiguousarray(inputs_kv.transpose(0, 2, 1)).astype(
        ml_dtypes.bfloat16
    )
    biasT = np.ascontiguousarray(bias[0].transpose(0, 2, 1))  # [H, lk, lq]
    if BIAS_DT == "bf16":
        biasT = biasT.astype(ml_dtypes.bfloat16)

    in_maps = []
    for bg in range(NB):
        bsl = slice(bg * BPC, (bg + 1) * BPC)
        for hg in range(NH):
            hsl = slice(hg * HPC, (hg + 1) * HPC)
            csl = slice(hg * HD, (hg + 1) * HD)
            in_maps.append(
                {
                    "xq_t": xqT[bsl],
                    "xk_t": xkT[bsl],
                    "bias_t": biasT[hsl],
                    "wq": np.ascontiguousarray(wq[:, csl]).astype(ml_dtypes.bfloat16),
                    "wk": np.ascontiguousarray(wk[:, csl]).astype(ml_dtypes.bfloat16),
                    "wv": np.ascontiguousarray(wv[:, csl]).astype(ml_dtypes.bfloat16),
                    "wo": np.ascontiguousarray(wo[csl, :]).astype(ml_dtypes.bfloat16),
                    "bq": np.ascontiguousarray(bq[csl]),
                    "bk": np.ascontiguousarray(bk[csl]),
                    "bv": np.ascontiguousarray(bv[csl]).astype(ml_dtypes.bfloat16),
                }
            )
    return in_maps


def assemble(results, bo):
    out = np.zeros((B, L, D), np.float32)
    for bg in range(NB):
        for hg in range(NH):
            out[bg * BPC:(bg + 1) * BPC] += results[bg * NH + hg]["out_part"]
    out += np.asarray(bo, np.float32)
    return out


def get_nc():
    if "nc" not in _CACHED:
        _CACHED["nc"] = _build_bass()
    return _CACHED["nc"]


def kernel(inputs_q, inputs_kv, bias, wq, bq, wk, bk, wv, bv, wo, bo):
    in_maps = make_in_maps(
        inputs_q, inputs_kv, bias, wq, bq, wk, bk, wv, bv, wo, bo
    )
    res = bass_utils.run_bass_kernel_spmd(
        get_nc(), in_maps, core_ids=list(range(8))
    )
    return assemble(res.results, bo)


# revision 18
# speedup vs baseline: 1.3740x; 1.3740x over previous
"""Trainium2 Bass kernel for nn_MultiHeadDotProductAttention (b=4, L=2048,
d_model=1024, 16 heads x 64 head_dim, additive attention bias, softmax).

Sharding: 8 cores = 2 batch-groups (2 batches each) x 4 head-groups (4 heads
each). Each core computes, for its 2 batches and 4 heads, the full attention
pipeline and an output-projection PARTIAL (summed over its 4 heads); the host
sums the 4 head-group partials per batch and adds the output bias.

Device layout ("T layout"): everything keeps sequence-length on the free dim
and feature dims on partitions, so no on-device transposes are needed:
  qT,kT: [hd, l]   from  out = wq^T @ xT  (xT transposed on host)
  logitsT[lk, lq] = kT^T-slices (K=64 contraction, two heads row-packed in the
                    128x128 PE array via tile_position)
  softmax over lk: exp on ACT; denominators via a ones-column appended to V in
                   the AV matmul (col-packed via tile_position); normalization
                   by reciprocal + gpsimd partition_broadcast + DVE multiply.
  out = ctxT^T @ wo with ctxT [hd, lq] directly produced by AV.

All matmuls run in float32r (tf32-class, 1 cycle/row at free-dim >= 256).
The attention bias is streamed as pre-transposed bf16 (biasT[h, lk, lq]) and
added to the fp32 logits in PSUM on the DVE.
"""

import numpy as np
from contextlib import ExitStack

import ml_dtypes

import concourse.bass as bass
import concourse.mybir as mybir
import concourse.tile as tile
from concourse import bacc
from concourse import bass_utils

F32 = mybir.dt.float32
F32R = mybir.dt.float32r
BF16 = mybir.dt.bfloat16
AF = mybir.ActivationFunctionType

# ---- problem constants (hardcoded per contract) ----
B, L, D = 4, 2048, 1024
H, DH = 16, 64
NB = 2          # batch groups (batches per core = B // NB = 2)
NH = 4          # head groups  (heads per core = H // NH = 4)
BPC = B // NB   # 2 batches per core
HPC = H // NH   # 4 heads per core
PAIRS = HPC // 2
KSUB = D // 128          # 8 contraction subtiles for projections
LCH = 256                # x-stream chunk width (free dim of projection mms)
NLC = L // LCH           # 8 chunks
NQ = 4                   # lq chunks of 512 for attention
NI = 16                  # lk chunks of 128
HD = HPC * DH            # 256 local head dims
HDC = HD // 128          # 2 local hd chunks (= PAIRS)

# bias dtype streamed to the device ("bf16" or "f32")
BIAS_DT = "bf16"
# fraction control: every PE_BIAS_EVERYth i-index adds bias on the PE via an
# identity matmul instead of the DVE (0 = never)
PE_BIAS_EVERY = 1

DEBUG_DUMPS = False

_CACHED = {}


def _build_bass():
    nc = bacc.Bacc("TRN2", target_bir_lowering=False, debug=False, num_devices=8)

    bias_dt = BF16 if BIAS_DT == "bf16" else F32R

    # ---- DRAM I/O (per core) ----
    xq_d = nc.dram_tensor("xq_t", [BPC, D, L], BF16, kind="ExternalInput")
    xk_d = nc.dram_tensor("xk_t", [BPC, D, L], BF16, kind="ExternalInput")
    bias_d = nc.dram_tensor("bias_t", [HPC, L, L], bias_dt, kind="ExternalInput")
    wq_d = nc.dram_tensor("wq", [D, HD], BF16, kind="ExternalInput")
    wk_d = nc.dram_tensor("wk", [D, HD], BF16, kind="ExternalInput")
    wv_d = nc.dram_tensor("wv", [D, HD], BF16, kind="ExternalInput")
    wo_d = nc.dram_tensor("wo", [HD, D], BF16, kind="ExternalInput")
    bq_d = nc.dram_tensor("bq", [HD], F32, kind="ExternalInput")
    bk_d = nc.dram_tensor("bk", [HD], F32, kind="ExternalInput")
    bv_d = nc.dram_tensor("bv", [HD], BF16, kind="ExternalInput")
    out_d = nc.dram_tensor("out_part", [BPC, L, D], F32, kind="ExternalOutput")
    if DEBUG_DUMPS:
        qT_d = nc.dram_tensor("qT_dbg", [128, HDC, BPC, L], F32R, kind="ExternalOutput")
        kT_d = nc.dram_tensor("kT_dbg", [128, HDC, BPC, L], F32R, kind="ExternalOutput")
        v_d = nc.dram_tensor("v_dbg", [128, NI, BPC, HPC, DH + 1], F32R, kind="ExternalOutput")
        ctxT_d = nc.dram_tensor("ctxT_dbg", [128, HDC, BPC, L], F32R, kind="ExternalOutput")
        exp_d = nc.dram_tensor("exp_dbg", [128, 512], F32R, kind="ExternalOutput")
        av_d = nc.dram_tensor("av_dbg", [128, 512], F32, kind="ExternalOutput")

    with tile.TileContext(nc) as tc, ExitStack() as top:
        # ---- persistent SBUF ----
        pers = top.enter_context(tc.tile_pool(name="pers", bufs=1))
        qT = pers.tile([128, HDC, BPC, L], BF16)
        kT = pers.tile([128, HDC, BPC, L], BF16)
        v = pers.tile([128, NI, BPC, HPC, DH + 1], BF16)
        ctxT = pers.tile([128, HDC, BPC, L], BF16)
        wo_s = pers.tile([128, HDC, D], BF16)
        bq_s = pers.tile([128, HDC], F32)
        bk_s = pers.tile([128, HDC], F32)
        bv_row = pers.tile([1, HD], BF16)
        ones_col = pers.tile([1, 128], BF16)
        ones_r = pers.tile([128, 128], F32R)
        ident_bf = pers.tile([128, 128], BF16)

        nc.sync.dma_start(wo_s[:], wo_d.rearrange("(c p) n -> p c n", p=128))
        nc.sync.dma_start(bq_s[:], bq_d.rearrange("(c p) -> p c", p=128))
        nc.sync.dma_start(bk_s[:], bk_d.rearrange("(c p) -> p c", p=128))
        nc.sync.dma_start(bv_row[:], bv_d[None, :])
        ones_f32 = pers.tile([128, 128], F32)
        nc.vector.memset(ones_f32[:], 1.0)
        nc.vector.tensor_copy(ones_col[:], ones_f32[0:1, :])
        nc.vector.tensor_copy(ones_r[:], ones_f32[:])
        from concourse.masks import make_identity
        ident_f32 = pers.tile([128, 128], F32)
        make_identity(nc, ident_f32[:])
        nc.vector.tensor_copy(ident_bf[:], ident_f32[:])
        # softmax-denominator column of v (column DH is all-ones)
        nc.vector.tensor_copy(
            v[:, :, :, :, DH],
            ones_f32[:, 0:NI * BPC * HPC].rearrange(
                "p (a b c) -> p a b c", a=NI, b=BPC
            ),
        )

        # ---- P1: projections ----
        with ExitStack() as p1:
            wpool = p1.enter_context(tc.tile_pool(name="wqkv", bufs=1))
            wq_s = wpool.tile([128, KSUB, HD], BF16)
            wk_s = wpool.tile([128, KSUB, HD], BF16)
            wv_s = wpool.tile([128, KSUB, HD], BF16)
            nc.sync.dma_start(wq_s[:], wq_d.rearrange("(k p) n -> p k n", p=128))
            nc.sync.dma_start(wk_s[:], wk_d.rearrange("(k p) n -> p k n", p=128))
            nc.sync.dma_start(wv_s[:], wv_d.rearrange("(k p) n -> p k n", p=128))

            xpool = p1.enter_context(tc.tile_pool(name="xs", bufs=3))
            psq = p1.enter_context(tc.tile_pool(name="psq", bufs=3, space="PSUM"))
            psv = p1.enter_context(tc.tile_pool(name="psv", bufs=2, space="PSUM"))

            for b in range(BPC):
                xqr = xq_d[b].rearrange("(k p) l -> p k l", p=128)
                xkr = xk_d[b].rearrange("(k p) l -> p k l", p=128)
                for c in range(NLC):
                    sl = slice(c * LCH, (c + 1) * LCH)
                    xq_t = xpool.tile([128, KSUB, LCH], BF16, tag="xq")
                    xk_t = xpool.tile([128, KSUB, LCH], BF16, tag="xk")
                    nc.sync.dma_start(xq_t[:], xqr[:, :, sl])
                    nc.sync.dma_start(xk_t[:], xkr[:, :, sl])
                    # qT / kT: out[hd-chunk(128), lq-chunk] = wq^T @ xT
                    for m in range(HDC):
                        msl = slice(m * 128, (m + 1) * 128)
                        pq = psq.tile([128, LCH], F32, tag="ps")
                        for k in range(KSUB):
                            nc.tensor.matmul(
                                pq[:], wq_s[:, k, msl], xq_t[:, k, :],
                                start=(k == 0), stop=(k == KSUB - 1),
                            )
                        nc.scalar.activation(
                            qT[:, m, b, sl], pq[:], AF.Identity, bias=bq_s[:, m:m + 1]
                        )
                        pk = psq.tile([128, LCH], F32, tag="ps")
                        for k in range(KSUB):
                            nc.tensor.matmul(
                                pk[:], wk_s[:, k, msl], xk_t[:, k, :],
                                start=(k == 0), stop=(k == KSUB - 1),
                            )
                        nc.scalar.activation(
                            kT[:, m, b, sl], pk[:], AF.Identity, bias=bk_s[:, m:m + 1]
                        )
                    # v: out[lk-sub(128), hd(256)] = xT-slices^T @ wv  (+ bv row)
                    for s in range(LCH // 128):
                        si = c * (LCH // 128) + s
                        pv = psv.tile([128, HD], F32, tag="ps")
                        for k in range(KSUB):
                            nc.tensor.matmul(
                                pv[:], xk_t[:, k, s * 128:(s + 1) * 128],
                                wv_s[:, k, :],
                                start=(k == 0), stop=False,
                            )
                        nc.tensor.matmul(
                            pv[:], ones_col[:], bv_row[:], start=False, stop=True
                        )
                        nc.vector.tensor_copy(
                            v[:, si, b, :, 0:DH],
                            pv[:].rearrange("p (h d) -> p h d", h=HPC),
                        )

        # ---- P2: attention ----
        with ExitStack() as p2:
            bpool = p2.enter_context(tc.tile_pool(name="biasb", bufs=8))
            epool = p2.enter_context(tc.tile_pool(name="expb", bufs=8))
            rpool = p2.enter_context(tc.tile_pool(name="recip", bufs=2))
            scpool = p2.enter_context(tc.tile_pool(name="scsh", bufs=2))
            reppool = p2.enter_context(tc.tile_pool(name="rep", bufs=2))
            pslg = p2.enter_context(tc.tile_pool(name="pslg", bufs=4, space="PSUM"))
            psav = p2.enter_context(tc.tile_pool(name="psav", bufs=4, space="PSUM"))

            for p in range(PAIRS):
                for n in range(NQ):
                    nsl = slice(n * 512, (n + 1) * 512)
                    av = {}
                    for hl in range(2):
                        for b in range(BPC):
                            av_t = psav.tile([128, 512], F32, tag="av")
                            av[hl, b] = av_t
                    for i in range(NI):
                        isl = slice(i * 128, (i + 1) * 128)
                        for hl in range(2):
                            h = p * 2 + hl
                            rsl = slice(hl * 64, (hl + 1) * 64)
                            bias_t = bpool.tile([128, 512], bias_dt, tag="bias")
                            nc.sync.dma_start(bias_t[:], bias_d[h, isl, nsl])
                            for b in range(BPC):
                                pe_bias = i % 3 == 0
                                lg = pslg.tile([128, 512], F32, tag="lg")
                                nc.tensor.matmul(
                                    lg[:],
                                    kT[rsl, p, b, isl],
                                    qT[rsl, p, b, nsl],
                                    start=True, stop=not pe_bias,
                                    tile_position=(hl * 64, 0),
                                )
                                if pe_bias:
                                    nc.tensor.matmul(
                                        lg[:], ident_bf[:], bias_t[:],
                                        start=False, stop=True,
                                        skip_group_check=True,
                                    )
                                else:
                                    nc.vector.tensor_add(lg[:], lg[:], bias_t[:])
                                et = epool.tile([128, 512], BF16, tag="exp")
                                nc.scalar.activation(et[:], lg[:], AF.Exp)
                                # AV accumulate: ctx rows + ones-row sums,
                                # col-packed so the pair lands at rows 0-63 /
                                # 64-127 of its psum (plus a sums row each).
                                if DEBUG_DUMPS and p == 0 and n == 0 and i == 0 and hl == 0 and b == 0:
                                    nc.sync.dma_start(exp_d[:], et[:])
                                nc.tensor.matmul(
                                    av[hl, b][0:DH + 1, :],
                                    v[:, i, b, 2 * p + hl, :],
                                    et[:],
                                    start=(i == 0), stop=(i == NI - 1),
                                )
                    if DEBUG_DUMPS and p == 0 and n == 0:
                        avdump = rpool.tile([128, 512], F32, tag="avd")
                        nc.vector.tensor_copy(avdump[:], av[0, 0][:])
                        nc.sync.dma_start(av_d[:], avdump[:])
                    # normalize -> ctxT: move sums row to SBUF, replicate
                    # across partitions with a K=1 ones matmul, reciprocal on
                    # DVE, then scale the ctx rows.
                    for hl in range(2):
                        for b in range(BPC):
                            rs = rpool.tile([128, 512], F32R, tag="rs")
                            nc.vector.tensor_copy(rs[64:65, :], av[hl, b][64:65, :])
                            repp = pslg.tile([128, 512], F32, tag="lg")
                            nc.tensor.matmul(
                                repp[:], ones_r[64:65, :], rs[64:65, :],
                                start=True, stop=True,
                            )
                            rep = reppool.tile([128, 512], F32, tag="rep")
                            rscr = reppool.tile([128, 512], F32, tag="rscr")
                            nc.vector.reciprocal_approx_accurate(
                                rep[0:64, :], repp[0:64, :], rscr[0:64, :]
                            )
                            if hl == 0:
                                nc.vector.tensor_mul(
                                    ctxT[0:64, p, b, nsl],
                                    av[hl, b][0:64, :],
                                    rep[0:64, :],
                                )
                            else:
                                sc = scpool.tile([64, 512], BF16, tag="sc")
                                nc.vector.tensor_mul(
                                    sc[:], av[hl, b][0:64, :], rep[0:64, :]
                                )
                                nc.sync.dma_start(ctxT[64:128, p, b, nsl], sc[:])

        if DEBUG_DUMPS:
            nc.sync.dma_start(qT_d[:], qT[:])
            nc.sync.dma_start(kT_d[:], kT[:])
            nc.sync.dma_start(v_d[:], v[:])
            nc.sync.dma_start(ctxT_d[:], ctxT[:])

        # ---- P3: output projection (partial over local heads) ----
        with ExitStack() as p3:
            opool = p3.enter_context(tc.tile_pool(name="outb", bufs=4))
            psout = p3.enter_context(tc.tile_pool(name="psout", bufs=2, space="PSUM"))
            for b in range(BPC):
                for m in range(L // 128):
                    msl = slice(m * 128, (m + 1) * 128)
                    for nn in range(D // 512):
                        osl = slice(nn * 512, (nn + 1) * 512)
                        po = psout.tile([128, 512], F32, tag="po")
                        for kc in range(HDC):
                            nc.tensor.matmul(
                                po[:],
                                ctxT[:, kc, b, msl],
                                wo_s[:, kc, osl],
                                start=(kc == 0), stop=(kc == HDC - 1),
                            )
                        ot = opool.tile([128, 512], F32, tag="ot")
                        nc.vector.tensor_copy(ot[:], po[:])
                        nc.sync.dma_start(out_d[b, msl, osl], ot[:])

    nc.compile()
    return nc


def make_in_maps(inputs_q, inputs_kv, bias, wq, bq, wk, bk, wv, bv, wo, bo):
    inputs_q = np.asarray(inputs_q, np.float32)
    inputs_kv = np.asarray(inputs_kv, np.float32)
    bias = np.asarray(bias, np.float32)
    wq = np.asarray(wq, np.float32).reshape(D, H * DH)
    wk = np.asarray(wk, np.float32).reshape(D, H * DH)
    wv = np.asarray(wv, np.float32).reshape(D, H * DH)
    bq = np.asarray(bq, np.float32).reshape(H * DH)
    bk = np.asarray(bk, np.float32).reshape(H * DH)
    bv = np.asarray(bv, np.float32).reshape(H * DH)
    wo = np.asarray(wo, np.float32).reshape(H * DH, D)
    bo = np.asarray(bo, np.float32)

    # fold the 1/sqrt(head_dim) query scaling into wq/bq
    s = 1.0 / np.sqrt(DH)
    wq = wq * s
    bq = bq * s

    # host-side layout marshalling for the chosen sharding
    xqT = np.ascontiguousarray(inputs_q.transpose(0, 2, 1)).astype(
        ml_dtypes.bfloat16
    )
    xkT = np.ascontiguousarray(inputs_kv.transpose(0, 2, 1)).astype(
        ml_dtypes.bfloat16
    )
    biasT = np.ascontiguousarray(bias[0].transpose(0, 2, 1))  # [H, lk, lq]
    if BIAS_DT == "bf16":
        biasT = biasT.astype(ml_dtypes.bfloat16)

    in_maps = []
    for bg in range(NB):
        bsl = slice(bg * BPC, (bg + 1) * BPC)
        for hg in range(NH):
            hsl = slice(hg * HPC, (hg + 1) * HPC)
            csl = slice(hg * HD, (hg + 1) * HD)
            in_maps.append(
                {
                    "xq_t": xqT[bsl],
                    "xk_t": xkT[bsl],
                    "bias_t": biasT[hsl],
                    "wq": np.ascontiguousarray(wq[:, csl]).astype(ml_dtypes.bfloat16),
                    "wk": np.ascontiguousarray(wk[:, csl]).astype(ml_dtypes.bfloat16),
                    "wv": np.ascontiguousarray(wv[:, csl]).astype(ml_dtypes.bfloat16),
                    "wo": np.ascontiguousarray(wo[csl, :]).astype(ml_dtypes.bfloat16),
                    "bq": np.ascontiguousarray(bq[csl]),
                    "bk": np.ascontiguousarray(bk[csl]),
                    "bv": np.ascontiguousarray(bv[csl]).astype(ml_dtypes.bfloat16),
                }
            )
    return in_maps


def assemble(results, bo):
    out = np.zeros((B, L, D), np.float32)
    for bg in range(NB):
        for hg in range(NH):
            out[bg * BPC:(bg + 1) * BPC] += results[bg * NH + hg]["out_part"]
    out += np.asarray(bo, np.float32)
    return out


def get_nc():
    if "nc" not in _CACHED:
        _CACHED["nc"] = _build_bass()
    return _CACHED["nc"]


def kernel(inputs_q, inputs_kv, bias, wq, bq, wk, bk, wv, bv, wo, bo):
    in_maps = make_in_maps(
        inputs_q, inputs_kv, bias, wq, bq, wk, bk, wv, bv, wo, bo
    )
    res = bass_utils.run_bass_kernel_spmd(
        get_nc(), in_maps, core_ids=list(range(8))
    )
    return assemble(res.results, bo)


# revision 19
# speedup vs baseline: 1.4696x; 1.0695x over previous
"""Trainium2 Bass kernel for nn_MultiHeadDotProductAttention (b=4, L=2048,
d_model=1024, 16 heads x 64 head_dim, additive attention bias, softmax).

Sharding: 8 cores = 2 batch-groups (2 batches each) x 4 head-groups (4 heads
each). Each core computes, for its 2 batches and 4 heads, the full attention
pipeline and an output-projection PARTIAL (summed over its 4 heads); the host
sums the 4 head-group partials per batch and adds the output bias.

Device layout ("T layout"): everything keeps sequence-length on the free dim
and feature dims on partitions, so no on-device transposes are needed:
  qT,kT: [hd, l]   from  out = wq^T @ xT  (xT transposed on host)
  logitsT[lk, lq] = kT^T-slices (K=64 contraction, two heads row-packed in the
                    128x128 PE array via tile_position)
  softmax over lk: exp on ACT; denominators via a ones-column appended to V in
                   the AV matmul (col-packed via tile_position); normalization
                   by reciprocal + gpsimd partition_broadcast + DVE multiply.
  out = ctxT^T @ wo with ctxT [hd, lq] directly produced by AV.

All matmuls run in float32r (tf32-class, 1 cycle/row at free-dim >= 256).
The attention bias is streamed as pre-transposed bf16 (biasT[h, lk, lq]) and
added to the fp32 logits in PSUM on the DVE.
"""

import numpy as np
from contextlib import ExitStack

import ml_dtypes

import concourse.bass as bass
import concourse.mybir as mybir
import concourse.tile as tile
from concourse import bacc
from concourse import bass_utils

F32 = mybir.dt.float32
F32R = mybir.dt.float32r
BF16 = mybir.dt.bfloat16
AF = mybir.ActivationFunctionType

# ---- problem constants (hardcoded per contract) ----
B, L, D = 4, 2048, 1024
H, DH = 16, 64
NB = 2          # batch groups (batches per core = B // NB = 2)
NH = 4          # head groups  (heads per core = H // NH = 4)
BPC = B // NB   # 2 batches per core
HPC = H // NH   # 4 heads per core
PAIRS = HPC // 2
KSUB = D // 128          # 8 contraction subtiles for projections
LCH = 256                # x-stream chunk width (free dim of projection mms)
NLC = L // LCH           # 8 chunks
NQ = 4                   # lq chunks of 512 for attention
NI = 16                  # lk chunks of 128
HD = HPC * DH            # 256 local head dims
HDC = HD // 128          # 2 local hd chunks (= PAIRS)

# bias dtype streamed to the device ("bf16" or "f32")
BIAS_DT = "bf16"
# fraction control: every PE_BIAS_EVERYth i-index adds bias on the PE via an
# identity matmul instead of the DVE (0 = never)
PE_BIAS_EVERY = 1

DEBUG_DUMPS = False

_CACHED = {}


def _build_bass():
    nc = bacc.Bacc("TRN2", target_bir_lowering=False, debug=False, num_devices=8)

    bias_dt = BF16 if BIAS_DT == "bf16" else F32R

    # ---- DRAM I/O (per core) ----
    xq_d = nc.dram_tensor("xq_t", [BPC, D, L], BF16, kind="ExternalInput")
    xk_d = nc.dram_tensor("xk_t", [BPC, D, L], BF16, kind="ExternalInput")
    bias_d = nc.dram_tensor("bias_t", [HPC, L, L], bias_dt, kind="ExternalInput")
    wq_d = nc.dram_tensor("wq", [D, HD], BF16, kind="ExternalInput")
    wk_d = nc.dram_tensor("wk", [D, HD], BF16, kind="ExternalInput")
    wv_d = nc.dram_tensor("wv", [D, HD], BF16, kind="ExternalInput")
    wo_d = nc.dram_tensor("wo", [HD, D], BF16, kind="ExternalInput")
    bq_d = nc.dram_tensor("bq", [HD], F32, kind="ExternalInput")
    bk_d = nc.dram_tensor("bk", [HD], F32, kind="ExternalInput")
    bv_d = nc.dram_tensor("bv", [HD], BF16, kind="ExternalInput")
    out_d = nc.dram_tensor("out_part", [BPC, L, D], F32, kind="ExternalOutput")
    if DEBUG_DUMPS:
        qT_d = nc.dram_tensor("qT_dbg", [128, HDC, BPC, L], F32R, kind="ExternalOutput")
        kT_d = nc.dram_tensor("kT_dbg", [128, HDC, BPC, L], F32R, kind="ExternalOutput")
        v_d = nc.dram_tensor("v_dbg", [128, NI, BPC, HPC, DH + 1], F32R, kind="ExternalOutput")
        ctxT_d = nc.dram_tensor("ctxT_dbg", [128, HDC, BPC, L], F32R, kind="ExternalOutput")
        exp_d = nc.dram_tensor("exp_dbg", [128, 512], F32R, kind="ExternalOutput")
        av_d = nc.dram_tensor("av_dbg", [128, 512], F32, kind="ExternalOutput")

    with tile.TileContext(nc) as tc, ExitStack() as top:
        # ---- persistent SBUF ----
        pers = top.enter_context(tc.tile_pool(name="pers", bufs=1))
        qT = pers.tile([128, HDC, BPC, L], BF16)
        kT = pers.tile([128, HDC, BPC, L], BF16)
        v = pers.tile([128, NI, BPC, HPC, DH + 1], BF16)
        ctxT = pers.tile([128, HDC, BPC, L], BF16)
        wo_s = pers.tile([128, HDC, D], BF16)
        bq_s = pers.tile([128, HDC], F32)
        bk_s = pers.tile([128, HDC], F32)
        bv_row = pers.tile([1, HD], BF16)
        ones_col = pers.tile([1, 128], BF16)
        ones_r = pers.tile([128, 128], F32R)
        ident_bf = pers.tile([128, 128], BF16)

        nc.sync.dma_start(wo_s[:], wo_d.rearrange("(c p) n -> p c n", p=128))
        nc.sync.dma_start(bq_s[:], bq_d.rearrange("(c p) -> p c", p=128))
        nc.sync.dma_start(bk_s[:], bk_d.rearrange("(c p) -> p c", p=128))
        nc.sync.dma_start(bv_row[:], bv_d[None, :])
        ones_f32 = pers.tile([128, 128], F32)
        nc.vector.memset(ones_f32[:], 1.0)
        nc.vector.tensor_copy(ones_col[:], ones_f32[0:1, :])
        nc.vector.tensor_copy(ones_r[:], ones_f32[:])
        from concourse.masks import make_identity
        ident_f32 = pers.tile([128, 128], F32)
        make_identity(nc, ident_f32[:])
        nc.vector.tensor_copy(ident_bf[:], ident_f32[:])
        # softmax-denominator column of v (column DH is all-ones)
        nc.vector.tensor_copy(
            v[:, :, :, :, DH],
            ones_f32[:, 0:NI * BPC * HPC].rearrange(
                "p (a b c) -> p a b c", a=NI, b=BPC
            ),
        )

        # ---- P1: projections ----
        with ExitStack() as p1:
            wpool = p1.enter_context(tc.tile_pool(name="wqkv", bufs=1))
            wq_s = wpool.tile([128, KSUB, HD], BF16)
            wk_s = wpool.tile([128, KSUB, HD], BF16)
            wv_s = wpool.tile([128, KSUB, HD], BF16)
            nc.sync.dma_start(wq_s[:], wq_d.rearrange("(k p) n -> p k n", p=128))
            nc.sync.dma_start(wk_s[:], wk_d.rearrange("(k p) n -> p k n", p=128))
            nc.sync.dma_start(wv_s[:], wv_d.rearrange("(k p) n -> p k n", p=128))

            xpool = p1.enter_context(tc.tile_pool(name="xs", bufs=2))
            psq = p1.enter_context(tc.tile_pool(name="psq", bufs=3, space="PSUM"))
            psv = p1.enter_context(tc.tile_pool(name="psv", bufs=2, space="PSUM"))

            for b in range(BPC):
                xqr = xq_d[b].rearrange("(k p) l -> p k l", p=128)
                xkr = xk_d[b].rearrange("(k p) l -> p k l", p=128)
                for c in range(NLC):
                    sl = slice(c * LCH, (c + 1) * LCH)
                    xq_t = xpool.tile([128, KSUB, LCH], BF16, tag="xq")
                    xk_t = xpool.tile([128, KSUB, LCH], BF16, tag="xk")
                    nc.sync.dma_start(xq_t[:], xqr[:, :, sl])
                    nc.sync.dma_start(xk_t[:], xkr[:, :, sl])
                    # qT / kT: out[hd-chunk(128), lq-chunk] = wq^T @ xT
                    for m in range(HDC):
                        msl = slice(m * 128, (m + 1) * 128)
                        pq = psq.tile([128, LCH], F32, tag="ps")
                        for k in range(KSUB):
                            nc.tensor.matmul(
                                pq[:], wq_s[:, k, msl], xq_t[:, k, :],
                                start=(k == 0), stop=(k == KSUB - 1),
                            )
                        nc.scalar.activation(
                            qT[:, m, b, sl], pq[:], AF.Identity, bias=bq_s[:, m:m + 1]
                        )
                        pk = psq.tile([128, LCH], F32, tag="ps")
                        for k in range(KSUB):
                            nc.tensor.matmul(
                                pk[:], wk_s[:, k, msl], xk_t[:, k, :],
                                start=(k == 0), stop=(k == KSUB - 1),
                            )
                        nc.scalar.activation(
                            kT[:, m, b, sl], pk[:], AF.Identity, bias=bk_s[:, m:m + 1]
                        )
                    # v: out[lk-sub(128), hd(256)] = xT-slices^T @ wv  (+ bv row)
                    for s in range(LCH // 128):
                        si = c * (LCH // 128) + s
                        pv = psv.tile([128, HD], F32, tag="ps")
                        for k in range(KSUB):
                            nc.tensor.matmul(
                                pv[:], xk_t[:, k, s * 128:(s + 1) * 128],
                                wv_s[:, k, :],
                                start=(k == 0), stop=False,
                            )
                        nc.tensor.matmul(
                            pv[:], ones_col[:], bv_row[:], start=False, stop=True
                        )
                        nc.vector.tensor_copy(
                            v[:, si, b, :, 0:DH],
                            pv[:].rearrange("p (h d) -> p h d", h=HPC),
                        )

        # ---- P2: attention ----
        with ExitStack() as p2:
            bpool = p2.enter_context(tc.tile_pool(name="biasb", bufs=6))
            epool = p2.enter_context(tc.tile_pool(name="expb", bufs=6))
            rpool = p2.enter_context(tc.tile_pool(name="recip", bufs=2))
            scpool = p2.enter_context(tc.tile_pool(name="scsh", bufs=2))
            reppool = p2.enter_context(tc.tile_pool(name="rep", bufs=2))
            pslg = p2.enter_context(tc.tile_pool(name="pslg", bufs=3, space="PSUM"))
            psav = p2.enter_context(tc.tile_pool(name="psav", bufs=4, space="PSUM"))
            psrep = p2.enter_context(tc.tile_pool(name="psrep", bufs=1, space="PSUM"))

            for p in range(PAIRS):
                for n in range(NQ):
                    nsl = slice(n * 512, (n + 1) * 512)
                    av = {}
                    for hl in range(2):
                        for b in range(BPC):
                            av_t = psav.tile([128, 512], F32, tag="av")
                            av[hl, b] = av_t
                    for i in range(NI):
                        isl = slice(i * 128, (i + 1) * 128)
                        for hl in range(2):
                            h = p * 2 + hl
                            rsl = slice(hl * 64, (hl + 1) * 64)
                            bias_t = bpool.tile([128, 512], bias_dt, tag="bias")
                            nc.sync.dma_start(bias_t[:], bias_d[h, isl, nsl])
                            for b in range(BPC):
                                pe_bias = i % 3 == 0
                                lg = pslg.tile([128, 512], F32, tag="lg")
                                nc.tensor.matmul(
                                    lg[:],
                                    kT[rsl, p, b, isl],
                                    qT[rsl, p, b, nsl],
                                    start=True, stop=not pe_bias,
                                    tile_position=(hl * 64, 0),
                                )
                                if pe_bias:
                                    nc.tensor.matmul(
                                        lg[:], ident_bf[:], bias_t[:],
                                        start=False, stop=True,
                                        skip_group_check=True,
                                    )
                                else:
                                    nc.vector.tensor_add(lg[:], lg[:], bias_t[:])
                                et = epool.tile([128, 512], BF16, tag="exp")
                                nc.scalar.activation(et[:], lg[:], AF.Exp)
                                # AV accumulate: ctx rows + ones-row sums,
                                # col-packed so the pair lands at rows 0-63 /
                                # 64-127 of its psum (plus a sums row each).
                                if DEBUG_DUMPS and p == 0 and n == 0 and i == 0 and hl == 0 and b == 0:
                                    nc.sync.dma_start(exp_d[:], et[:])
                                nc.tensor.matmul(
                                    av[hl, b][0:DH + 1, :],
                                    v[:, i, b, 2 * p + hl, :],
                                    et[:],
                                    start=(i == 0), stop=(i == NI - 1),
                                )
                    if DEBUG_DUMPS and p == 0 and n == 0:
                        avdump = rpool.tile([128, 512], F32, tag="avd")
                        nc.vector.tensor_copy(avdump[:], av[0, 0][:])
                        nc.sync.dma_start(av_d[:], avdump[:])
                    # normalize -> ctxT: move sums row to SBUF, replicate
                    # across partitions with a K=1 ones matmul, reciprocal on
                    # DVE, then scale the ctx rows.
                    for hl in range(2):
                        for b in range(BPC):
                            rs = rpool.tile([128, 512], F32R, tag="rs")
                            nc.vector.tensor_copy(rs[64:65, :], av[hl, b][64:65, :])
                            repp = psrep.tile([128, 512], F32, tag="repp")
                            nc.tensor.matmul(
                                repp[:], ones_r[64:65, :], rs[64:65, :],
                                start=True, stop=True,
                            )
                            rep = reppool.tile([128, 512], F32, tag="rep")
                            rscr = reppool.tile([128, 512], F32, tag="rscr")
                            nc.vector.reciprocal_approx_accurate(
                                rep[0:64, :], repp[0:64, :], rscr[0:64, :]
                            )
                            if hl == 0:
                                nc.vector.tensor_mul(
                                    ctxT[0:64, p, b, nsl],
                                    av[hl, b][0:64, :],
                                    rep[0:64, :],
                                )
                            else:
                                sc = scpool.tile([64, 512], BF16, tag="sc")
                                nc.vector.tensor_mul(
                                    sc[:], av[hl, b][0:64, :], rep[0:64, :]
                                )
                                nc.sync.dma_start(ctxT[64:128, p, b, nsl], sc[:])

        if DEBUG_DUMPS:
            nc.sync.dma_start(qT_d[:], qT[:])
            nc.sync.dma_start(kT_d[:], kT[:])
            nc.sync.dma_start(v_d[:], v[:])
            nc.sync.dma_start(ctxT_d[:], ctxT[:])

        # ---- P3: output projection (partial over local heads) ----
        with ExitStack() as p3:
            opool = p3.enter_context(tc.tile_pool(name="outb", bufs=4))
            psout = p3.enter_context(tc.tile_pool(name="psout", bufs=2, space="PSUM"))
            for b in range(BPC):
                for m in range(L // 128):
                    msl = slice(m * 128, (m + 1) * 128)
                    for nn in range(D // 512):
                        osl = slice(nn * 512, (nn + 1) * 512)
                        po = psout.tile([128, 512], F32, tag="po")
                        for kc in range(HDC):
                            nc.tensor.matmul(
                                po[:],
                                ctxT[:, kc, b, msl],
                                wo_s[:, kc, osl],
                                start=(kc == 0), stop=(kc == HDC - 1),
                            )
                        ot = opool.tile([128, 512], F32, tag="ot")
                        nc.scalar.copy(ot[:], po[:])
                        nc.sync.dma_start(out_d[b, msl, osl], ot[:])

    nc.compile()
    return nc


def make_in_maps(inputs_q, inputs_kv, bias, wq, bq, wk, bk, wv, bv, wo, bo):
    inputs_q = np.asarray(inputs_q, np.float32)
    inputs_kv = np.asarray(inputs_kv, np.float32)
    bias = np.asarray(bias, np.float32)
    wq = np.asarray(wq, np.float32).reshape(D, H * DH)
    wk = np.asarray(wk, np.float32).reshape(D, H * DH)
    wv = np.asarray(wv, np.float32).reshape(D, H * DH)
    bq = np.asarray(bq, np.float32).reshape(H * DH)
    bk = np.asarray(bk, np.float32).reshape(H * DH)
    bv = np.asarray(bv, np.float32).reshape(H * DH)
    wo = np.asarray(wo, np.float32).reshape(H * DH, D)
    bo = np.asarray(bo, np.float32)

    # fold the 1/sqrt(head_dim) query scaling into wq/bq
    s = 1.0 / np.sqrt(DH)
    wq = wq * s
    bq = bq * s

    # host-side layout marshalling for the chosen sharding
    xqT = np.ascontiguousarray(inputs_q.transpose(0, 2, 1)).astype(
        ml_dtypes.bfloat16
    )
    xkT = np.ascontiguousarray(inputs_kv.transpose(0, 2, 1)).astype(
        ml_dtypes.bfloat16
    )
    biasT = np.ascontiguousarray(bias[0].transpose(0, 2, 1))  # [H, lk, lq]
    if BIAS_DT == "bf16":
        biasT = biasT.astype(ml_dtypes.bfloat16)

    in_maps = []
    for bg in range(NB):
        bsl = slice(bg * BPC, (bg + 1) * BPC)
        for hg in range(NH):
            hsl = slice(hg * HPC, (hg + 1) * HPC)
            csl = slice(hg * HD, (hg + 1) * HD)
            in_maps.append(
                {
                    "xq_t": xqT[bsl],
                    "xk_t": xkT[bsl],
                    "bias_t": biasT[hsl],
                    "wq": np.ascontiguousarray(wq[:, csl]).astype(ml_dtypes.bfloat16),
                    "wk": np.ascontiguousarray(wk[:, csl]).astype(ml_dtypes.bfloat16),
                    "wv": np.ascontiguousarray(wv[:, csl]).astype(ml_dtypes.bfloat16),
                    "wo": np.ascontiguousarray(wo[csl, :]).astype(ml_dtypes.bfloat16),
                    "bq": np.ascontiguousarray(bq[csl]),
                    "bk": np.ascontiguousarray(bk[csl]),
                    "bv": np.ascontiguousarray(bv[csl]).astype(ml_dtypes.bfloat16),
                }
            )
    return in_maps


def assemble(results, bo):
    out = np.zeros((B, L, D), np.float32)
    for bg in range(NB):
        for hg in range(NH):
            out[bg * BPC:(bg + 1) * BPC] += results[bg * NH + hg]["out_part"]
    out += np.asarray(bo, np.float32)
    return out


def get_nc():
    if "nc" not in _CACHED:
        _CACHED["nc"] = _build_bass()
    return _CACHED["nc"]


def kernel(inputs_q, inputs_kv, bias, wq, bq, wk, bk, wv, bv, wo, bo):
    in_maps = make_in_maps(
        inputs_q, inputs_kv, bias, wq, bq, wk, bk, wv, bv, wo, bo
    )
    res = bass_utils.run_bass_kernel_spmd(
        get_nc(), in_maps, core_ids=list(range(8))
    )
    return assemble(res.results, bo)


# revision 20
# speedup vs baseline: 1.7939x; 1.2207x over previous
"""Trainium2 Bass kernel for nn_MultiHeadDotProductAttention (b=4, L=2048,
d_model=1024, 16 heads x 64 head_dim, additive attention bias, softmax).

Sharding: 8 cores = 2 batch-groups (2 batches each) x 4 head-groups (4 heads
each). Each core computes, for its 2 batches and 4 heads, the full attention
pipeline and an output-projection PARTIAL (summed over its 4 heads); the host
sums the 4 head-group partials per batch and adds the output bias.

Device layout ("T layout"): everything keeps sequence-length on the free dim
and feature dims on partitions, so no on-device transposes are needed:
  qT,kT: [hd, l]   from  out = wq^T @ xT  (xT transposed on host)
  logitsT[lk, lq] = kT^T-slices (K=64 contraction, two heads row-packed in the
                    128x128 PE array via tile_position)
  softmax over lk: exp on ACT; denominators via a ones-column appended to V in
                   the AV matmul (col-packed via tile_position); normalization
                   by reciprocal + gpsimd partition_broadcast + DVE multiply.
  out = ctxT^T @ wo with ctxT [hd, lq] directly produced by AV.

All matmuls run in float32r (tf32-class, 1 cycle/row at free-dim >= 256).
The attention bias is streamed as pre-transposed bf16 (biasT[h, lk, lq]) and
added to the fp32 logits in PSUM on the DVE.
"""

import numpy as np
from contextlib import ExitStack

import ml_dtypes

import concourse.bass as bass
import concourse.mybir as mybir
import concourse.tile as tile
from concourse import bacc
from concourse import bass_utils

F32 = mybir.dt.float32
F32R = mybir.dt.float32r
BF16 = mybir.dt.bfloat16
AF = mybir.ActivationFunctionType

# ---- problem constants (hardcoded per contract) ----
B, L, D = 4, 2048, 1024
H, DH = 16, 64
NB = 2          # batch groups (batches per core = B // NB = 2)
NH = 4          # head groups  (heads per core = H // NH = 4)
BPC = B // NB   # 2 batches per core
HPC = H // NH   # 4 heads per core
PAIRS = HPC // 2
KSUB = D // 128          # 8 contraction subtiles for projections
LCH = 256                # x-stream chunk width (free dim of projection mms)
NLC = L // LCH           # 8 chunks
NQ = 4                   # lq chunks of 512 for attention
NI = 16                  # lk chunks of 128
HD = HPC * DH            # 256 local head dims
HDC = HD // 128          # 2 local hd chunks (= PAIRS)

# bias dtype streamed to the device ("bf16" or "f32")
BIAS_DT = "bf16"
# fraction control: every PE_BIAS_EVERYth i-index adds bias on the PE via an
# identity matmul instead of the DVE (0 = never)
PE_BIAS_EVERY = 1

DEBUG_DUMPS = False

_CACHED = {}


def _build_bass():
    nc = bacc.Bacc("TRN2", target_bir_lowering=False, debug=False, num_devices=8)

    bias_dt = BF16 if BIAS_DT == "bf16" else F32R

    # ---- DRAM I/O (per core) ----
    xq_d = nc.dram_tensor("xq_t", [BPC, D, L], BF16, kind="ExternalInput")
    xk_d = nc.dram_tensor("xk_t", [BPC, D, L], BF16, kind="ExternalInput")
    bias_d = nc.dram_tensor("bias_t", [HPC, L, L], bias_dt, kind="ExternalInput")
    wq_d = nc.dram_tensor("wq", [D, HD], BF16, kind="ExternalInput")
    wk_d = nc.dram_tensor("wk", [D, HD], BF16, kind="ExternalInput")
    wv_d = nc.dram_tensor("wv", [D, HD], BF16, kind="ExternalInput")
    wo_d = nc.dram_tensor("wo", [HD, D], BF16, kind="ExternalInput")
    bq_d = nc.dram_tensor("bq", [HD], F32, kind="ExternalInput")
    bk_d = nc.dram_tensor("bk", [HD], F32, kind="ExternalInput")
    bv_d = nc.dram_tensor("bv", [HD], BF16, kind="ExternalInput")
    out_d = nc.dram_tensor("out_part", [BPC, L, D], F32, kind="ExternalOutput")
    if DEBUG_DUMPS:
        qT_d = nc.dram_tensor("qT_dbg", [128, HDC, BPC, L], F32R, kind="ExternalOutput")
        kT_d = nc.dram_tensor("kT_dbg", [128, HDC, BPC, L], F32R, kind="ExternalOutput")
        v_d = nc.dram_tensor("v_dbg", [128, NI, BPC, HPC, DH + 1], F32R, kind="ExternalOutput")
        ctxT_d = nc.dram_tensor("ctxT_dbg", [128, HDC, BPC, L], F32R, kind="ExternalOutput")
        exp_d = nc.dram_tensor("exp_dbg", [128, 512], F32R, kind="ExternalOutput")
        av_d = nc.dram_tensor("av_dbg", [128, 512], F32, kind="ExternalOutput")

    with tile.TileContext(nc) as tc, ExitStack() as top:
        # ---- persistent SBUF ----
        pers = top.enter_context(tc.tile_pool(name="pers", bufs=1))
        qT = pers.tile([128, HDC, BPC, L], BF16)
        kT = pers.tile([128, HDC, BPC, L], BF16)
        v = pers.tile([128, NI, BPC, HPC, DH + 1], BF16)
        ctxT = pers.tile([128, HDC, BPC, L], BF16)
        wo_s = pers.tile([128, HDC, D], BF16)
        bq_s = pers.tile([128, HDC], F32)
        bk_s = pers.tile([128, HDC], F32)
        bv_row = pers.tile([1, HD], BF16)
        ones_col = pers.tile([1, 128], BF16)
        ones_r = pers.tile([128, 128], F32R)
        ident_bf = pers.tile([128, 128], BF16)

        nc.sync.dma_start(wo_s[:], wo_d.rearrange("(c p) n -> p c n", p=128))
        nc.sync.dma_start(bq_s[:], bq_d.rearrange("(c p) -> p c", p=128))
        nc.sync.dma_start(bk_s[:], bk_d.rearrange("(c p) -> p c", p=128))
        nc.sync.dma_start(bv_row[:], bv_d[None, :])
        ones_f32 = pers.tile([128, 128], F32)
        nc.vector.memset(ones_f32[:], 1.0)
        nc.vector.tensor_copy(ones_col[:], ones_f32[0:1, :])
        nc.vector.tensor_copy(ones_r[:], ones_f32[:])
        from concourse.masks import make_identity
        ident_f32 = pers.tile([128, 128], F32)
        make_identity(nc, ident_f32[:])
        nc.vector.tensor_copy(ident_bf[:], ident_f32[:])
        # softmax-denominator column of v (column DH is all-ones)
        nc.vector.tensor_copy(
            v[:, :, :, :, DH],
            ones_f32[:, 0:NI * BPC * HPC].rearrange(
                "p (a b c) -> p a b c", a=NI, b=BPC
            ),
        )

        # ---- P1: projections ----
        with ExitStack() as p1:
            wpool = p1.enter_context(tc.tile_pool(name="wqkv", bufs=1))
            wq_s = wpool.tile([128, KSUB, HD], BF16)
            wk_s = wpool.tile([128, KSUB, HD], BF16)
            wv_s = wpool.tile([128, KSUB, HD], BF16)
            nc.sync.dma_start(wq_s[:], wq_d.rearrange("(k p) n -> p k n", p=128))
            nc.sync.dma_start(wk_s[:], wk_d.rearrange("(k p) n -> p k n", p=128))
            nc.sync.dma_start(wv_s[:], wv_d.rearrange("(k p) n -> p k n", p=128))

            xpool = p1.enter_context(tc.tile_pool(name="xs", bufs=2))
            psq = p1.enter_context(tc.tile_pool(name="psq", bufs=3, space="PSUM"))
            psv = p1.enter_context(tc.tile_pool(name="psv", bufs=2, space="PSUM"))

            for b in range(BPC):
                xqr = xq_d[b].rearrange("(k p) l -> p k l", p=128)
                xkr = xk_d[b].rearrange("(k p) l -> p k l", p=128)
                for c in range(NLC):
                    sl = slice(c * LCH, (c + 1) * LCH)
                    xq_t = xpool.tile([128, KSUB, LCH], BF16, tag="xq")
                    xk_t = xpool.tile([128, KSUB, LCH], BF16, tag="xk")
                    nc.sync.dma_start(xq_t[:], xqr[:, :, sl])
                    nc.sync.dma_start(xk_t[:], xkr[:, :, sl])
                    # qT / kT: out[hd-chunk(128), lq-chunk] = wq^T @ xT
                    for m in range(HDC):
                        msl = slice(m * 128, (m + 1) * 128)
                        pq = psq.tile([128, LCH], F32, tag="ps")
                        for k in range(KSUB):
                            nc.tensor.matmul(
                                pq[:], wq_s[:, k, msl], xq_t[:, k, :],
                                start=(k == 0), stop=(k == KSUB - 1),
                            )
                        nc.scalar.activation(
                            qT[:, m, b, sl], pq[:], AF.Identity, bias=bq_s[:, m:m + 1]
                        )
                        pk = psq.tile([128, LCH], F32, tag="ps")
                        for k in range(KSUB):
                            nc.tensor.matmul(
                                pk[:], wk_s[:, k, msl], xk_t[:, k, :],
                                start=(k == 0), stop=(k == KSUB - 1),
                            )
                        nc.scalar.activation(
                            kT[:, m, b, sl], pk[:], AF.Identity, bias=bk_s[:, m:m + 1]
                        )
                    # v: out[lk-sub(128), hd(256)] = xT-slices^T @ wv  (+ bv row)
                    for s in range(LCH // 128):
                        si = c * (LCH // 128) + s
                        pv = psv.tile([128, HD], F32, tag="ps")
                        for k in range(KSUB):
                            nc.tensor.matmul(
                                pv[:], xk_t[:, k, s * 128:(s + 1) * 128],
                                wv_s[:, k, :],
                                start=(k == 0), stop=False,
                            )
                        nc.tensor.matmul(
                            pv[:], ones_col[:], bv_row[:], start=False, stop=True
                        )
                        nc.vector.tensor_copy(
                            v[:, si, b, :, 0:DH],
                            pv[:].rearrange("p (h d) -> p h d", h=HPC),
                        )

        # ---- P2: attention ----
        with ExitStack() as p2:
            bpool = p2.enter_context(tc.tile_pool(name="biasb", bufs=6))
            epool = p2.enter_context(tc.tile_pool(name="expb", bufs=6))
            lgspool = p2.enter_context(tc.tile_pool(name="lgs", bufs=8))
            rpool = p2.enter_context(tc.tile_pool(name="recip", bufs=2))
            scpool = p2.enter_context(tc.tile_pool(name="scsh", bufs=2))
            reppool = p2.enter_context(tc.tile_pool(name="rep", bufs=2))
            pslg = p2.enter_context(tc.tile_pool(name="pslg", bufs=3, space="PSUM"))
            psav = p2.enter_context(tc.tile_pool(name="psav", bufs=4, space="PSUM"))
            psrep = p2.enter_context(tc.tile_pool(name="psrep", bufs=1, space="PSUM"))

            for p in range(PAIRS):
                for n in range(NQ):
                    nsl = slice(n * 512, (n + 1) * 512)
                    av = {}
                    for hl in range(2):
                        for b in range(BPC):
                            av_t = psav.tile([128, 512], F32, tag="av")
                            av[hl, b] = av_t
                    for i in range(NI):
                        isl = slice(i * 128, (i + 1) * 128)
                        for hl in range(2):
                            h = p * 2 + hl
                            rsl = slice(hl * 64, (hl + 1) * 64)
                            bias_t = bpool.tile([128, 512], bias_dt, tag="bias")
                            nc.sync.dma_start(bias_t[:], bias_d[h, isl, nsl])
                            for b in range(BPC):
                                pe_bias = i % 3 == 0
                                lg = pslg.tile([128, 512], F32, tag="lg")
                                nc.tensor.matmul(
                                    lg[:],
                                    kT[rsl, p, b, isl],
                                    qT[rsl, p, b, nsl],
                                    start=True, stop=not pe_bias,
                                    tile_position=(hl * 64, 0),
                                )
                                if pe_bias:
                                    nc.tensor.matmul(
                                        lg[:], ident_bf[:], bias_t[:],
                                        start=False, stop=True,
                                        skip_group_check=True,
                                    )
                                    et = epool.tile([128, 512], BF16, tag="exp")
                                    nc.scalar.activation(et[:], lg[:], AF.Exp)
                                else:
                                    # add bias to SBUF (not in-place) so the
                                    # PSUM bank frees at the DVE, not the ACT
                                    lgs = lgspool.tile([128, 512], F32, tag="lgs")
                                    nc.vector.tensor_add(lgs[:], lg[:], bias_t[:])
                                    et = epool.tile([128, 512], BF16, tag="exp")
                                    nc.scalar.activation(et[:], lgs[:], AF.Exp)
                                # AV accumulate: ctx rows + ones-row sums,
                                # col-packed so the pair lands at rows 0-63 /
                                # 64-127 of its psum (plus a sums row each).
                                if DEBUG_DUMPS and p == 0 and n == 0 and i == 0 and hl == 0 and b == 0:
                                    nc.sync.dma_start(exp_d[:], et[:])
                                nc.tensor.matmul(
                                    av[hl, b][0:DH + 1, :],
                                    v[:, i, b, 2 * p + hl, :],
                                    et[:],
                                    start=(i == 0), stop=(i == NI - 1),
                                )
                    if DEBUG_DUMPS and p == 0 and n == 0:
                        avdump = rpool.tile([128, 512], F32, tag="avd")
                        nc.vector.tensor_copy(avdump[:], av[0, 0][:])
                        nc.sync.dma_start(av_d[:], avdump[:])
                    # normalize -> ctxT: move sums row to SBUF, replicate
                    # across partitions with a K=1 ones matmul, reciprocal on
                    # DVE, then scale the ctx rows.
                    for hl in range(2):
                        for b in range(BPC):
                            rs = rpool.tile([128, 512], F32R, tag="rs")
                            nc.vector.tensor_copy(rs[64:65, :], av[hl, b][64:65, :])
                            repp = psrep.tile([128, 512], F32, tag="repp")
                            nc.tensor.matmul(
                                repp[:], ones_r[64:65, :], rs[64:65, :],
                                start=True, stop=True,
                            )
                            rep = reppool.tile([128, 512], F32, tag="rep")
                            rscr = reppool.tile([128, 512], F32, tag="rscr")
                            nc.vector.reciprocal_approx_accurate(
                                rep[0:64, :], repp[0:64, :], rscr[0:64, :]
                            )
                            if hl == 0:
                                nc.vector.tensor_mul(
                                    ctxT[0:64, p, b, nsl],
                                    av[hl, b][0:64, :],
                                    rep[0:64, :],
                                )
                            else:
                                sc = scpool.tile([64, 512], BF16, tag="sc")
                                nc.vector.tensor_mul(
                                    sc[:], av[hl, b][0:64, :], rep[0:64, :]
                                )
                                nc.sync.dma_start(ctxT[64:128, p, b, nsl], sc[:])

        if DEBUG_DUMPS:
            nc.sync.dma_start(qT_d[:], qT[:])
            nc.sync.dma_start(kT_d[:], kT[:])
            nc.sync.dma_start(v_d[:], v[:])
            nc.sync.dma_start(ctxT_d[:], ctxT[:])

        # ---- P3: output projection (partial over local heads) ----
        with ExitStack() as p3:
            opool = p3.enter_context(tc.tile_pool(name="outb", bufs=4))
            psout = p3.enter_context(tc.tile_pool(name="psout", bufs=2, space="PSUM"))
            for b in range(BPC):
                for m in range(L // 128):
                    msl = slice(m * 128, (m + 1) * 128)
                    for nn in range(D // 512):
                        osl = slice(nn * 512, (nn + 1) * 512)
                        po = psout.tile([128, 512], F32, tag="po")
                        for kc in range(HDC):
                            nc.tensor.matmul(
                                po[:],
                                ctxT[:, kc, b, msl],
                                wo_s[:, kc, osl],
                                start=(kc == 0), stop=(kc == HDC - 1),
                            )
                        ot = opool.tile([128, 512], F32, tag="ot")
                        nc.scalar.copy(ot[:], po[:])
                        nc.sync.dma_start(out_d[b, msl, osl], ot[:])

    nc.compile()
    return nc


def make_in_maps(inputs_q, inputs_kv, bias, wq, bq, wk, bk, wv, bv, wo, bo):
    inputs_q = np.asarray(inputs_q, np.float32)
    inputs_kv = np.asarray(inputs_kv, np.float32)
    bias = np.asarray(bias, np.float32)
    wq = np.asarray(wq, np.float32).reshape(D, H * DH)
    wk = np.asarray(wk, np.float32).reshape(D, H * DH)
    wv = np.asarray(wv, np.float32).reshape(D, H * DH)
    bq = np.asarray(bq, np.float32).reshape(H * DH)
    bk = np.asarray(bk, np.float32).reshape(H * DH)
    bv = np.asarray(bv, np.float32).reshape(H * DH)
    wo = np.asarray(wo, np.float32).reshape(H * DH, D)
    bo = np.asarray(bo, np.float32)

    # fold the 1/sqrt(head_dim) query scaling into wq/bq
    s = 1.0 / np.sqrt(DH)
    wq = wq * s
    bq = bq * s

    # host-side layout marshalling for the chosen sharding
    xqT = np.ascontiguousarray(inputs_q.transpose(0, 2, 1)).astype(
        ml_dtypes.bfloat16
    )
    xkT = np.ascontiguousarray(inputs_kv.transpose(0, 2, 1)).astype(
        ml_dtypes.bfloat16
    )
    biasT = np.ascontiguousarray(bias[0].transpose(0, 2, 1))  # [H, lk, lq]
    if BIAS_DT == "bf16":
        biasT = biasT.astype(ml_dtypes.bfloat16)

    in_maps = []
    for bg in range(NB):
        bsl = slice(bg * BPC, (bg + 1) * BPC)
        for hg in range(NH):
            hsl = slice(hg * HPC, (hg + 1) * HPC)
            csl = slice(hg * HD, (hg + 1) * HD)
            in_maps.append(
                {
                    "xq_t": xqT[bsl],
                    "xk_t": xkT[bsl],
                    "bias_t": biasT[hsl],
                    "wq": np.ascontiguousarray(wq[:, csl]).astype(ml_dtypes.bfloat16),
                    "wk": np.ascontiguousarray(wk[:, csl]).astype(ml_dtypes.bfloat16),
                    "wv": np.ascontiguousarray(wv[:, csl]).astype(ml_dtypes.bfloat16),
                    "wo": np.ascontiguousarray(wo[csl, :]).astype(ml_dtypes.bfloat16),
                    "bq": np.ascontiguousarray(bq[csl]),
                    "bk": np.ascontiguousarray(bk[csl]),
                    "bv": np.ascontiguousarray(bv[csl]).astype(ml_dtypes.bfloat16),
                }
            )
    return in_maps


def assemble(results, bo):
    out = np.zeros((B, L, D), np.float32)
    for bg in range(NB):
        for hg in range(NH):
            out[bg * BPC:(bg + 1) * BPC] += results[bg * NH + hg]["out_part"]
    out += np.asarray(bo, np.float32)
    return out


def get_nc():
    if "nc" not in _CACHED:
        _CACHED["nc"] = _build_bass()
    return _CACHED["nc"]


def kernel(inputs_q, inputs_kv, bias, wq, bq, wk, bk, wv, bv, wo, bo):
    in_maps = make_in_maps(
        inputs_q, inputs_kv, bias, wq, bq, wk, bk, wv, bv, wo, bo
    )
    res = bass_utils.run_bass_kernel_spmd(
        get_nc(), in_maps, core_ids=list(range(8))
    )
    return assemble(res.results, bo)
